# revision 1
# baseline (speedup 1.0000x reference)
"""Trainium2 Bass kernel for the non-local attention block (nn_CPP_80676665688885).

Sharding: pure data-parallel over batch — 1 sample per NeuronCore (B=8, 8 cores).
BatchNorm batch-statistics are combined with a tiny (2 KB) AllGather.

fp32 matmuls on TRN2 run in LOW_HIGH mode (2 passes, ~2.5 cyc/col) — ~5x the
cost of bf16. So every large matmul here is decomposed into bf16 passes:
  exact-ish (error ~2^-16): A@B = A_hi@B_hi + A_hi@B_lo + A_lo@B_hi
  where X_hi = bf16(X), X_lo = bf16(X - X_hi); fp32 accumulation in PSUM.
exp(fT) is written directly as bf16: its quantization acts as a correlated
perturbation of softmax logits (numerator and denominator use the same
values), so the final error stays ~1e-4 relative.

Per-core algorithm (sample x: (C=256, N=4096), N = 64x64 spatial):
  theta = Wt@x + bt  (split hi/lo)     phi,g = maxpool2(conv)  (phi split, g
  transposed then split)
  fT    = phi^T @ theta  3 bf16 passes; exp on ScalarE -> expf bf16
  y     = gT^T @ expf    2 bf16 passes (gT hi/lo), accumulated over m-chunks
  s[n]  = ones^T @ expf  1 bf16 pass,  accumulated over m-chunks
  y_n   = y * (1/s)  (reciprocal exactly on (128,x) layout via DRAM bounce)
  wy    = Ww @ y_n   (native fp32; bias bw dropped — cancels in BatchNorm)
  S1,S2 per channel -> AllGather over 8 cores -> local sum
  z     = (wy - mean)*rsqrt(var+eps)*gamma + beta + x ; out = max_n z
"""

import numpy as np
from contextlib import ExitStack

import concourse.bass as bass
import concourse.bacc as bacc
import concourse.tile as tile
from concourse import mybir
from concourse.bass_utils import run_bass_kernel_spmd

F32 = mybir.dt.float32
BF16 = mybir.dt.bfloat16
AF = mybir.ActivationFunctionType
ALU = mybir.AluOpType
AX = mybir.AxisListType

B = 8
C = 256
CI = 128
N = 4096          # 64*64
M = 1024          # 32*32 after 2x2 maxpool
NT = 512          # n-tile (PSUM bank width in fp32)
NTILES = N // NT  # 8
MCH = M // 128    # 8 m-chunks
CCH = C // 128    # 2 channel chunks
EPS = 1e-5
INV_CNT = 1.0 / (B * N)

_CACHE = {}


def _build():
    nc = bacc.Bacc("TRN2", num_devices=B)

    x_d = nc.declare_dram_parameter("x", [C, N], F32, False)
    # hi/lo bf16-split projection weights, pre-transposed host-side
    w_hi_d = {}
    w_lo_d = {}
    for nm in ("t", "p", "g"):
        w_hi_d[nm] = nc.declare_dram_parameter(f"W{nm}Thi", [C, CI], BF16, False)
        w_lo_d[nm] = nc.declare_dram_parameter(f"W{nm}Tlo", [C, CI], BF16, False)
    wwT_hi_d = nc.declare_dram_parameter("WwThi", [CI, C], BF16, False)
    wwT_lo_d = nc.declare_dram_parameter("WwTlo", [CI, C], BF16, False)
    bt_d = nc.declare_dram_parameter("bt", [CI, 1], F32, False)
    bp_d = nc.declare_dram_parameter("bp", [CI, 1], F32, False)
    bg_d = nc.declare_dram_parameter("bg", [CI, 1], F32, False)
    gamma_d = nc.declare_dram_parameter("gamma", [128, CCH], F32, False)
    beta_d = nc.declare_dram_parameter("beta", [128, CCH], F32, False)
    out_d = nc.declare_dram_parameter("out", [CCH, 128], F32, True)

    ident_d = nc.inline_tensor(np.eye(128, dtype=np.float32), name="ident")

    # DRAM bounce buffers
    s_dram = nc.dram_tensor("s_bounce", [1, N], F32)
    r_hi_dram = nc.dram_tensor("r_hi_bounce", [1, N], BF16)
    r_lo_dram = nc.dram_tensor("r_lo_bounce", [1, N], BF16)
    warm_in = nc.dram_tensor("warm_in", [1, 8], F32)
    warm_out = nc.dram_tensor("warm_out", [1, 8], F32, addr_space="Shared")
    stats_in = nc.dram_tensor("stats_in", [128, 2 * CCH], F32)
    stats_out = nc.dram_tensor("stats_out", [128, 2 * CCH], F32,
                               addr_space="Shared")

    with ExitStack() as ctx:
        tc = ctx.enter_context(tile.TileContext(nc))
        consts = ctx.enter_context(tc.tile_pool(name="consts", bufs=1))
        persist = ctx.enter_context(tc.tile_pool(name="persist", bufs=1))
        scratch = ctx.enter_context(tc.tile_pool(name="scratch", bufs=2))
        efp = ctx.enter_context(tc.tile_pool(name="efp", bufs=5))
        small = ctx.enter_context(tc.tile_pool(name="small", bufs=4))
        ps_ft = ctx.enter_context(tc.tile_pool(name="ps_ft", bufs=2, space="PSUM"))
        ps_y = ctx.enter_context(tc.tile_pool(name="ps_y", bufs=2, space="PSUM"))
        ps_s = ctx.enter_context(tc.tile_pool(name="ps_s", bufs=1, space="PSUM"))
        ps_rb = ctx.enter_context(tc.tile_pool(name="ps_rb", bufs=1, space="PSUM"))
        ps_cv = ctx.enter_context(tc.tile_pool(name="ps_cv", bufs=2, space="PSUM"))

        # ---- constants / weights into SBUF ----
        ident = consts.tile([128, 128], F32)
        nc.sync.dma_start(out=ident, in_=ident_d[:, :])
        ones_k = consts.tile([128, 1], BF16)
        nc.vector.memset(ones_k, 1.0)
        ones_p = consts.tile([1, 128], BF16)
        nc.vector.memset(ones_p, 1.0)
        eps_sb = consts.tile([128, 1], F32)
        nc.vector.memset(eps_sb, EPS)

        w_hi = {}
        w_lo = {}
        for nm in ("t", "p", "g"):
            w_hi[nm] = consts.tile([128, CCH, CI], BF16, name=f"w_hi_{nm}")
            w_lo[nm] = consts.tile([128, CCH, CI], BF16, name=f"w_lo_{nm}")
            for ch in range(CCH):
                cs = slice(ch * 128, (ch + 1) * 128)
                nc.sync.dma_start(out=w_hi[nm][:, ch, :], in_=w_hi_d[nm][cs, :])
                nc.sync.dma_start(out=w_lo[nm][:, ch, :], in_=w_lo_d[nm][cs, :])
        ww_hi = consts.tile([128, CCH, 128], BF16)
        ww_lo = consts.tile([128, CCH, 128], BF16)
        for ch in range(CCH):
            nc.sync.dma_start(out=ww_hi[:, ch, :], in_=wwT_hi_d[:, ch * 128:(ch + 1) * 128])
            nc.sync.dma_start(out=ww_lo[:, ch, :], in_=wwT_lo_d[:, ch * 128:(ch + 1) * 128])
        bt_sb = consts.tile([128, 1], F32)
        bp_sb = consts.tile([128, 1], F32)
        bg_sb = consts.tile([128, 1], F32)
        nc.sync.dma_start(out=bt_sb, in_=bt_d[:, :])
        nc.sync.dma_start(out=bp_sb, in_=bp_d[:, :])
        nc.sync.dma_start(out=bg_sb, in_=bg_d[:, :])
        gamma_sb = consts.tile([128, CCH], F32)
        beta_sb = consts.tile([128, CCH], F32)
        nc.sync.dma_start(out=gamma_sb, in_=gamma_d[:, :])
        nc.sync.dma_start(out=beta_sb, in_=beta_d[:, :])

        # warm up the collective path early (overlaps with compute)
        warm_sb = small.tile([1, 8], F32, tag="warm")
        nc.vector.memset(warm_sb, 1.0)
        nc.sync.dma_start(out=warm_in[:, :], in_=warm_sb)
        nc.gpsimd.collective_compute(
            "AllReduce", ALU.add, replica_groups=[list(range(B))],
            ins=[warm_in[:, :]], outs=[warm_out[:, :]])

        # ---- x into SBUF, split hi/lo ----
        x_sb = [persist.tile([128, N], F32, tag=f"x{ch}", name=f"x_sb{ch}")
                for ch in range(CCH)]
        x_hi = [persist.tile([128, N], BF16, tag=f"xh{ch}", name=f"x_hi{ch}")
                for ch in range(CCH)]
        x_lo = [persist.tile([128, N], BF16, tag=f"xl{ch}", name=f"x_lo{ch}")
                for ch in range(CCH)]
        for ch in range(CCH):
            nc.sync.dma_start(out=x_sb[ch], in_=x_d[ch * 128:(ch + 1) * 128, :])
            nc.scalar.copy(out=x_hi[ch], in_=x_sb[ch])
            nc.vector.tensor_tensor(out=x_lo[ch], in0=x_sb[ch], in1=x_hi[ch],
                                    op=ALU.subtract)

        # ---- projections (3-term bf16 conv) ----
        # theta: kept as hi/lo bf16 tiles; phi/g: fp32 for pooling
        th_hi = persist.tile([128, N], BF16, tag="thh")
        th_lo = persist.tile([128, N], BF16, tag="thl")
        phi_full = scratch.tile([128, N], F32, tag="s4")
        g_full = scratch.tile([128, N], F32, tag="s4")

        def conv_mms(ps, nm, sl):
            terms = ((w_hi[nm], x_hi), (w_hi[nm], x_lo), (w_lo[nm], x_hi))
            nterm = len(terms) * CCH
            k = 0
            for ch in range(CCH):
                for lhs, rhs in terms:
                    nc.tensor.matmul(ps, lhsT=lhs[:, ch, :], rhs=rhs[ch][:, sl],
                                     start=(k == 0), stop=(k == nterm - 1))
                    k += 1

        for it in range(NTILES):
            sl = slice(it * NT, (it + 1) * NT)
            ps = ps_cv.tile([128, NT], F32, tag="cv")
            conv_mms(ps, "t", sl)
            # theta + bias, split hi/lo (hi on ScalarE, lo on VectorE)
            nc.scalar.activation(out=th_hi[:, sl], in_=ps, func=AF.Identity,
                                 bias=bt_sb, scale=1.0)
            nc.vector.scalar_tensor_tensor(out=th_lo[:, sl], in0=ps, scalar=bt_sb,
                                           in1=th_hi[:, sl], op0=ALU.add,
                                           op1=ALU.subtract)
        for dst, nm, b_sb in ((phi_full, "p", bp_sb), (g_full, "g", bg_sb)):
            for it in range(NTILES):
                sl = slice(it * NT, (it + 1) * NT)
                ps = ps_cv.tile([128, NT], F32, tag="cv")
                conv_mms(ps, nm, sl)
                nc.vector.tensor_scalar_add(out=dst[:, sl], in0=ps, scalar1=b_sb)

        # ---- 2x2 maxpool on phi and g ----
        phi_pool = persist.tile([128, M], F32, tag="phip")
        g_pool = persist.tile([128, M], F32, tag="gp")
        pp1 = scratch.tile([128, 64 * 32], F32, tag="pool1")
        gp1 = scratch.tile([128, 64 * 32], F32, tag="pool1")
        for src, mid, dst in ((phi_full, pp1, phi_pool), (g_full, gp1, g_pool)):
            sr = src.rearrange("p (h wp t) -> p h wp t", h=64, wp=32, t=2)
            nc.vector.tensor_tensor(
                out=mid.rearrange("p (h wp) -> p h wp", h=64),
                in0=sr[:, :, :, 0], in1=sr[:, :, :, 1], op=ALU.max)
            mr = mid.rearrange("p (hp s wp) -> p hp s wp", hp=32, s=2, wp=32)
            nc.vector.tensor_tensor(
                out=dst.rearrange("p (hp wp) -> p hp wp", hp=32),
                in0=mr[:, :, 0, :], in1=mr[:, :, 1, :], op=ALU.max)

        # phi hi/lo split
        phi_hi = persist.tile([128, M], BF16, tag="phih")
        phi_lo = persist.tile([128, M], BF16, tag="phil")
        nc.scalar.copy(out=phi_hi, in_=phi_pool)
        nc.vector.tensor_tensor(out=phi_lo, in0=phi_pool, in1=phi_hi,
                                op=ALU.subtract)

        # ---- transpose g_pool (CI, M) -> gT chunks (m=128, CI), split hi/lo ----
        gT32 = persist.tile([128, MCH, CI], F32, tag="gT32")
        gT_hi = persist.tile([128, MCH, CI], BF16, tag="gTh")
        gT_lo = persist.tile([128, MCH, CI], BF16, tag="gTl")
        for mc in range(MCH):
            tp = ps_cv.tile([128, 128], F32, tag="cv")
            nc.tensor.transpose(tp, g_pool[:, mc * 128:(mc + 1) * 128], ident)
            nc.scalar.copy(out=gT32[:, mc, :], in_=tp)
            nc.scalar.copy(out=gT_hi[:, mc, :], in_=gT32[:, mc, :])
            nc.vector.tensor_tensor(out=gT_lo[:, mc, :], in0=gT32[:, mc, :],
                                    in1=gT_hi[:, mc, :], op=ALU.subtract)

        # ---- attention + normalization + W-conv, per n-tile ----
        y_hi = persist.tile([128, N], BF16, tag="ynh")
        y_lo = persist.tile([128, N], BF16, tag="ynl")
        wy = [scratch.tile([128, N], F32, tag="s4", name=f"wy{ch}")
              for ch in range(CCH)]
        rT = persist.tile([128, NTILES * (NT // 128)], F32, tag="rT")
        s1p = persist.tile([128, CCH, NTILES], F32, tag="s1p")
        s2p = persist.tile([128, CCH, NTILES], F32, tag="s2p")

        for it in range(NTILES):
            sl = slice(it * NT, (it + 1) * NT)
            yps = ps_y.tile([128, NT], F32, tag="yps")
            sps = ps_s.tile([1, NT], F32, tag="sps")
            for mc in range(MCH):
                ms = slice(mc * 128, (mc + 1) * 128)
                fps = ps_ft.tile([128, NT], F32, tag="ft")
                nc.tensor.matmul(fps, lhsT=phi_hi[:, ms], rhs=th_hi[:, sl],
                                 start=True, stop=False)
                nc.tensor.matmul(fps, lhsT=phi_hi[:, ms], rhs=th_lo[:, sl],
                                 start=False, stop=False)
                nc.tensor.matmul(fps, lhsT=phi_lo[:, ms], rhs=th_hi[:, sl],
                                 start=False, stop=True)
                ef = efp.tile([128, NT], BF16, tag="ef")
                nc.scalar.activation(out=ef, in_=fps, func=AF.Exp)
                nc.tensor.matmul(yps, lhsT=gT_hi[:, mc, :], rhs=ef,
                                 start=(mc == 0), stop=False)
                nc.tensor.matmul(yps, lhsT=gT_lo[:, mc, :], rhs=ef,
                                 start=False, stop=(mc == MCH - 1))
                nc.tensor.matmul(sps, lhsT=ones_k, rhs=ef,
                                 start=(mc == 0), stop=(mc == MCH - 1))

            # s -> SBUF, bounce via DRAM into (128, NT/128) layout, recip, back
            s_sb = small.tile([1, NT], F32, tag="s1d")
            nc.scalar.copy(out=s_sb, in_=sps)
            nc.sync.dma_start(out=s_dram[:, sl], in_=s_sb)
            f4 = NT // 128
            sl4 = slice(it * f4, (it + 1) * f4)
            sT_t = small.tile([128, f4], F32, tag="sT")
            nc.sync.dma_start(out=sT_t, in_=s_dram[0, sl].rearrange("(p f) -> p f", p=128))
            nc.vector.reciprocal(out=rT[:, sl4], in_=sT_t)
            rT_hi = small.tile([128, f4], BF16, tag="rTh")
            rT_lo = small.tile([128, f4], BF16, tag="rTl")
            nc.vector.tensor_copy(out=rT_hi, in_=rT[:, sl4])
            nc.vector.tensor_tensor(out=rT_lo, in0=rT[:, sl4], in1=rT_hi, op=ALU.subtract)
            nc.sync.dma_start(out=r_hi_dram[0, sl].rearrange("(p f) -> p f", p=128), in_=rT_hi)
            nc.sync.dma_start(out=r_lo_dram[0, sl].rearrange("(p f) -> p f", p=128), in_=rT_lo)
            r_hi_sb = small.tile([1, NT], BF16, tag="r1dh")
            r_lo_sb = small.tile([1, NT], BF16, tag="r1dl")
            nc.sync.dma_start(out=r_hi_sb, in_=r_hi_dram[:, sl])
            nc.sync.dma_start(out=r_lo_sb, in_=r_lo_dram[:, sl])

            # broadcast r across partitions via two K=1 bf16 matmuls
            rbps = ps_rb.tile([128, NT], F32, tag="rb")
            nc.tensor.matmul(rbps, lhsT=ones_p, rhs=r_hi_sb, start=True, stop=False)
            nc.tensor.matmul(rbps, lhsT=ones_p, rhs=r_lo_sb, start=False, stop=True)
            rb_sb = small.tile([128, NT], F32, tag="rb_sb")
            nc.scalar.copy(out=rb_sb, in_=rbps)
            nc.vector.scalar_tensor_tensor(
                out=y_hi[:, sl], in0=yps, scalar=1.0, in1=rb_sb,
                op0=ALU.mult, op1=ALU.mult)
            # y_lo = y - y_hi = (yps*rb) - y_hi
            nc.vector.scalar_tensor_tensor(
                out=y_lo[:, sl], in0=yps, scalar=1.0, in1=rb_sb,
                op0=ALU.mult, op1=ALU.mult)
            nc.vector.tensor_tensor(out=y_lo[:, sl], in0=y_lo[:, sl],
                                    in1=y_hi[:, sl], op=ALU.subtract)

            # W conv (3-term bf16); accumulate BN partial stats
            for ch in range(CCH):
                wps = ps_cv.tile([128, NT], F32, tag="cv")
                nc.tensor.matmul(wps, lhsT=ww_hi[:, ch, :], rhs=y_hi[:, sl],
                                 start=True, stop=False)
                nc.tensor.matmul(wps, lhsT=ww_hi[:, ch, :], rhs=y_lo[:, sl],
                                 start=False, stop=False)
                nc.tensor.matmul(wps, lhsT=ww_lo[:, ch, :], rhs=y_hi[:, sl],
                                 start=False, stop=True)
                nc.vector.tensor_scalar(
                    out=wy[ch][:, sl], in0=wps, scalar1=0.0, scalar2=None,
                    op0=ALU.add, op1=ALU.add, accum_out=s1p[:, ch, it:it + 1])
                sqt = efp.tile([128, NT], BF16, tag="sqtrash")
                nc.scalar.activation(
                    out=sqt, in_=wy[ch][:, sl], func=AF.Square,
                    accum_out=s2p[:, ch, it:it + 1])

        # ---- combine partials, AllGather, local sum, finalize ----
        stats_sb = small.tile([128, 2 * CCH], F32, tag="stats")
        for ch in range(CCH):
            nc.vector.tensor_reduce(out=stats_sb[:, 2 * ch:2 * ch + 1],
                                    in_=s1p[:, ch, :], axis=AX.X, op=ALU.add)
            nc.vector.tensor_reduce(out=stats_sb[:, 2 * ch + 1:2 * ch + 2],
                                    in_=s2p[:, ch, :], axis=AX.X, op=ALU.add)
        nc.sync.dma_start(out=stats_in[:, :], in_=stats_sb)
        nc.gpsimd.collective_compute(
            "AllReduce", ALU.add, replica_groups=[list(range(B))],
            ins=[stats_in[:, :]], outs=[stats_out[:, :]])
        stats_g = small.tile([128, 2 * CCH], F32, tag="statsg")
        nc.sync.dma_start(out=stats_g, in_=stats_out[:, :])

        out_sb = small.tile([128, CCH], F32, tag="outsb")
        for ch in range(CCH):
            mean = small.tile([128, 1], F32, tag="fin")
            e2 = small.tile([128, 1], F32, tag="fin")
            m2 = small.tile([128, 1], F32, tag="fin")
            var = small.tile([128, 1], F32, tag="fin")
            nc.vector.tensor_scalar_mul(out=mean, in0=stats_g[:, 2 * ch:2 * ch + 1],
                                        scalar1=INV_CNT)
            nc.vector.tensor_scalar_mul(out=e2, in0=stats_g[:, 2 * ch + 1:2 * ch + 2],
                                        scalar1=INV_CNT)
            nc.scalar.square(out=m2, in_=mean)
            nc.vector.tensor_tensor(out=var, in0=e2, in1=m2, op=ALU.subtract)
            sd = small.tile([128, 1], F32, tag="fin")
            nc.scalar.activation(out=sd, in_=var, func=AF.Sqrt, bias=eps_sb, scale=1.0)
            inv = small.tile([128, 1], F32, tag="fin")
            nc.vector.reciprocal(out=inv, in_=sd)
            scale = small.tile([128, 1], F32, tag="fin")
            nc.vector.tensor_tensor(out=scale, in0=inv, in1=gamma_sb[:, ch:ch + 1],
                                    op=ALU.mult)
            negshift = small.tile([128, 1], F32, tag="fin")
            nc.vector.scalar_tensor_tensor(
                out=negshift, in0=mean, scalar=scale, in1=beta_sb[:, ch:ch + 1],
                op0=ALU.mult, op1=ALU.subtract)
            # z' = wy*scale + x (in place over wy); out = max_n z' - negshift
            nc.vector.scalar_tensor_tensor(
                out=wy[ch][:, :], in0=wy[ch][:, :], scalar=scale, in1=x_sb[ch],
                op0=ALU.mult, op1=ALU.add)
            mx = small.tile([128, 1], F32, tag="fin")
            nc.vector.tensor_reduce(out=mx, in_=wy[ch][:, :], axis=AX.X, op=ALU.max)
            nc.vector.tensor_tensor(out=out_sb[:, ch:ch + 1], in0=mx, in1=negshift,
                                    op=ALU.subtract)
        for ch in range(CCH):
            nc.sync.dma_start(out=out_d[ch, :].rearrange("(p one) -> p one", one=1),
                              in_=out_sb[:, ch:ch + 1])

    nc.compile()
    return nc


_LAST = {}


def kernel(**inputs):
    x = np.ascontiguousarray(inputs["x"], dtype=np.float32)      # (8, 256, 64, 64)
    Wg = np.asarray(inputs["Wg"], dtype=np.float32)
    bg = np.asarray(inputs["bg"], dtype=np.float32)
    Wt = np.asarray(inputs["Wt"], dtype=np.float32)
    bt = np.asarray(inputs["bt"], dtype=np.float32)
    Wp = np.asarray(inputs["Wp"], dtype=np.float32)
    bp = np.asarray(inputs["bp"], dtype=np.float32)
    Ww = np.asarray(inputs["Ww"], dtype=np.float32)
    gamma = np.asarray(inputs["gamma"], dtype=np.float32)
    beta = np.asarray(inputs["beta"], dtype=np.float32)

    if "nc" not in _CACHE:
        _CACHE["nc"] = _build()
    nc = _CACHE["nc"]

    try:
        import ml_dtypes
        bf = ml_dtypes.bfloat16
    except ImportError:
        import jax.numpy as jnp
        bf = jnp.bfloat16

    def split(w):
        hi = np.ascontiguousarray(w.astype(bf))
        lo = np.ascontiguousarray((w - hi.astype(np.float32)).astype(bf))
        return hi, lo

    WtThi, WtTlo = split(np.ascontiguousarray(Wt.T))
    WpThi, WpTlo = split(np.ascontiguousarray(Wp.T))
    WgThi, WgTlo = split(np.ascontiguousarray(Wg.T))
    WwThi, WwTlo = split(np.ascontiguousarray(Ww.T))

    shared = {
        "WtThi": WtThi, "WtTlo": WtTlo,
        "WpThi": WpThi, "WpTlo": WpTlo,
        "WgThi": WgThi, "WgTlo": WgTlo,
        "WwThi": WwThi, "WwTlo": WwTlo,
        "bt": np.ascontiguousarray(bt.reshape(CI, 1)),
        "bp": np.ascontiguousarray(bp.reshape(CI, 1)),
        "bg": np.ascontiguousarray(bg.reshape(CI, 1)),
        "gamma": np.ascontiguousarray(gamma.reshape(CCH, 128).T),
        "beta": np.ascontiguousarray(beta.reshape(CCH, 128).T),
    }
    in_maps = [dict(shared, x=np.ascontiguousarray(x[b].reshape(C, N)))
               for b in range(B)]
    import os
    trace = bool(int(os.environ.get("KERNEL_TRACE", "0")))
    res = run_bass_kernel_spmd(nc, in_maps, core_ids=list(range(B)), trace=trace)
    _LAST["res"] = res
    out = np.stack([np.asarray(res.results[b]["out"]).reshape(C) for b in range(B)])
    return out.reshape(B, C, 1, 1).astype(np.float32)


if __name__ == "__main__":
    pass



# revision 7
# speedup vs baseline: 1.5137x; 1.5137x over previous
"""Trainium2 Bass kernel for the non-local attention block (nn_CPP_80676665688885).

Sharding: pure data-parallel over batch — 1 sample per NeuronCore (B=8, 8 cores).
BatchNorm batch-statistics are combined with a tiny (2 KB) AllReduce.

All matmuls run in float32r (single-pass, 1 col/cycle like bf16, ~13-bit
mantissa) — no hi/lo splitting needed anywhere. exp() output stays in f32r.
Softmax normalization (1/s) is deferred past the W-conv: (Ww@y)/s == Ww@(y/s),
so the per-tile critical chain is fT -> exp -> y -> wconv with the reciprocal
path (s -> 1/s -> partition-broadcast via K=1 matmul) running in parallel.

Per-core algorithm (sample x: (C=256, N=4096), N = 64x64 spatial):
  theta = Wt@x + bt                       phi,g = maxpool2(conv) + bias post-pool
  per n-tile (512 cols), per m-chunk (128):
    fT   = phi^T @ theta   (f32r)  ; ef = exp(fT) (f32r, ScalarE)
    y   += gT^T @ ef ; s += ones^T @ ef
  r = 1/s ; rb = broadcast(r) via ones(1,128)^T @ r matmul
  wy_n = (Ww @ y) * rb  -> fp16, with fused Σ (BN s1); Σwy² via gpsimd+vector
  stats AllReduce over 8 cores ; scale = gamma*rsqrt(var+eps) (rsqrt = exp(-½ln))
  out[c] = max_n(wy_n*scale + x) + (beta - mean*scale)
"""

import numpy as np
from contextlib import ExitStack

import concourse.bass as bass
import concourse.bacc as bacc
import concourse.tile as tile
from concourse import mybir
from concourse.bass_utils import run_bass_kernel_spmd

F32 = mybir.dt.float32
F32R = mybir.dt.float32r
F16 = mybir.dt.float16
BF16 = mybir.dt.bfloat16
AF = mybir.ActivationFunctionType
ALU = mybir.AluOpType
AX = mybir.AxisListType

B = 8
C = 256
CI = 128
N = 4096          # 64*64
M = 1024          # 32*32 after 2x2 maxpool
NT = 512          # n-tile (PSUM bank width in fp32)
NTILES = N // NT  # 8
MCH = M // 128    # 8 m-chunks
CCH = C // 128    # 2 channel chunks
EPS = 1e-5
INV_CNT = 1.0 / (B * N)

_CACHE = {}


def _build():
    nc = bacc.Bacc("TRN2", num_devices=B)

    x_d = nc.declare_dram_parameter("x", [C, N], F32R, False)
    w_d = {}
    for nm in ("t", "p", "g"):
        w_d[nm] = nc.declare_dram_parameter(f"W{nm}T", [C, CI], F32R, False)
    wwT_d = nc.declare_dram_parameter("WwT", [CI, C], F32R, False)
    bt_d = nc.declare_dram_parameter("bt", [CI, 1], F32, False)
    bp_d = nc.declare_dram_parameter("bp", [CI, 1], F32, False)
    bg_d = nc.declare_dram_parameter("bg", [CI, 1], F32, False)
    gamma_d = nc.declare_dram_parameter("gamma", [128, CCH], F32, False)
    beta_d = nc.declare_dram_parameter("beta", [128, CCH], F32, False)
    ones_k_d = nc.declare_dram_parameter("ones_k", [128, 1], F32R, False)
    ones_p_d = nc.declare_dram_parameter("ones_p", [1, 128], F32R, False)
    out_d = nc.declare_dram_parameter("out", [CCH, 128], F32, True)

    ident_d = nc.inline_tensor(np.eye(128, dtype=np.float32), name="ident")

    warm_in = nc.dram_tensor("warm_in", [1, 8], F32)
    warm_out = nc.dram_tensor("warm_out", [1, 8], F32, addr_space="Shared")
    stats_in = nc.dram_tensor("stats_in", [128, 2 * CCH], F32)
    stats_out = nc.dram_tensor("stats_out", [128, 2 * CCH], F32,
                               addr_space="Shared")

    with ExitStack() as ctx:
        tc = ctx.enter_context(tile.TileContext(nc))
        consts = ctx.enter_context(tc.tile_pool(name="consts", bufs=1))
        persist = ctx.enter_context(tc.tile_pool(name="persist", bufs=1))
        efp = ctx.enter_context(tc.tile_pool(name="efp", bufs=3))
        sm = ctx.enter_context(tc.tile_pool(name="sm", bufs=2))
        small = ctx.enter_context(tc.tile_pool(name="small", bufs=4))
        # PSUM: 8 banks total
        ps_cv = ctx.enter_context(tc.tile_pool(name="ps_cv", bufs=2, space="PSUM"))
        ps_ft = ctx.enter_context(tc.tile_pool(name="ps_ft", bufs=2, space="PSUM"))
        ps_y = ctx.enter_context(tc.tile_pool(name="ps_y", bufs=2, space="PSUM"))
        ps_s = ctx.enter_context(tc.tile_pool(name="ps_s", bufs=1, space="PSUM"))
        ps_rb = ctx.enter_context(tc.tile_pool(name="ps_rb", bufs=1, space="PSUM"))

        # ---- constants / weights into SBUF ----
        ident = consts.tile([128, 128], F32)
        nc.sync.dma_start(out=ident, in_=ident_d[:, :])
        ones_k = consts.tile([128, 1], F32R)
        nc.sync.dma_start(out=ones_k, in_=ones_k_d[:, :])
        ones_p = consts.tile([1, 128], F32R)
        nc.sync.dma_start(out=ones_p, in_=ones_p_d[:, :])
        eps_sb = consts.tile([128, 1], F32)
        nc.vector.memset(eps_sb, EPS)

        w_sb = {}
        for nm in ("t", "p", "g"):
            w_sb[nm] = consts.tile([128, CCH, CI], F32R, name=f"w_{nm}")
            for ch in range(CCH):
                nc.sync.dma_start(out=w_sb[nm][:, ch, :],
                                  in_=w_d[nm][ch * 128:(ch + 1) * 128, :])
        ww_sb = consts.tile([128, CCH, 128], F32R)
        for ch in range(CCH):
            nc.sync.dma_start(out=ww_sb[:, ch, :],
                              in_=wwT_d[:, ch * 128:(ch + 1) * 128])
        bt_sb = consts.tile([128, 1], F32)
        bp_sb = consts.tile([128, 1], F32)
        bg_sb = consts.tile([128, 1], F32)
        nc.sync.dma_start(out=bt_sb, in_=bt_d[:, :])
        nc.sync.dma_start(out=bp_sb, in_=bp_d[:, :])
        nc.sync.dma_start(out=bg_sb, in_=bg_d[:, :])
        gamma_sb = consts.tile([128, CCH], F32)
        beta_sb = consts.tile([128, CCH], F32)
        nc.sync.dma_start(out=gamma_sb, in_=gamma_d[:, :])
        nc.sync.dma_start(out=beta_sb, in_=beta_d[:, :])

        # warm up the collective path early (overlaps with compute)
        warm_sb = small.tile([1, 8], F32, tag="warm")
        nc.vector.memset(warm_sb, 1.0)
        nc.sync.dma_start(out=warm_in[:, :], in_=warm_sb)
        nc.gpsimd.collective_compute(
            "AllReduce", ALU.add, replica_groups=[list(range(B))],
            ins=[warm_in[:, :]], outs=[warm_out[:, :]])

        # ---- x into SBUF (chunked DMA so convs can start early) ----
        x_sb = [persist.tile([128, N], F32R, tag=f"x{ch}", name=f"x_sb{ch}")
                for ch in range(CCH)]
        QN = N // 4
        for q in range(4):
            qs = slice(q * QN, (q + 1) * QN)
            for ch in range(CCH):
                nc.sync.dma_start(out=x_sb[ch][:, qs],
                                  in_=x_d[ch * 128:(ch + 1) * 128, qs])

        # fp16 copy of x for the finale (vector, overlaps conv matmuls)
        x16 = [persist.tile([128, N], F16, tag=f"x16_{ch}", name=f"x16_{ch}")
               for ch in range(CCH)]

        # ---- projections ----
        th = persist.tile([128, N], F32R, tag="th")
        phi_pool = persist.tile([128, M], F32R, tag="phip")
        g_pool = persist.tile([128, M], F32, tag="gp")
        gT = persist.tile([128, MCH, CI], F32R, tag="gT")

        def conv_mms(ps, nm, sl):
            for ch in range(CCH):
                nc.tensor.matmul(ps, lhsT=w_sb[nm][:, ch, :], rhs=x_sb[ch][:, sl],
                                 start=(ch == 0), stop=(ch == CCH - 1))

        for it in range(NTILES):
            sl = slice(it * NT, (it + 1) * NT)
            ms = slice(it * 128, (it + 1) * 128)
            # theta
            ps = ps_cv.tile([128, NT], F32, tag="cv")
            conv_mms(ps, "t", sl)
            nc.vector.tensor_scalar_add(out=th[:, sl], in0=ps, scalar1=bt_sb)
            # phi conv -> copy -> 2x2 maxpool -> +bias
            psp = ps_cv.tile([128, NT], F32, tag="cv")
            conv_mms(psp, "p", sl)
            pcp = sm.tile([128, NT], F32, tag="pcp")
            nc.scalar.copy(out=pcp, in_=psp)
            pr = pcp.rearrange("p (h wp t) -> p h wp t", h=8, wp=32, t=2)
            pm = sm.tile([128, 8, 32], F32, tag="pm")
            nc.vector.tensor_tensor(out=pm, in0=pr[:, :, :, 0], in1=pr[:, :, :, 1],
                                    op=ALU.max)
            pm2 = pm.rearrange("p (hp s) wp -> p hp s wp", s=2)
            pp = sm.tile([128, 128], F32, tag="pp")
            nc.vector.tensor_tensor(
                out=pp.rearrange("p (hp wp) -> p hp wp", hp=4),
                in0=pm2[:, :, 0, :], in1=pm2[:, :, 1, :], op=ALU.max)
            nc.vector.tensor_scalar_add(out=phi_pool[:, ms], in0=pp,
                                        scalar1=bp_sb)
            # g conv -> copy -> maxpool -> +bias -> transpose
            psg = ps_cv.tile([128, NT], F32, tag="cv")
            conv_mms(psg, "g", sl)
            gcp = sm.tile([128, NT], F32, tag="gcp")
            nc.scalar.copy(out=gcp, in_=psg)
            gr = gcp.rearrange("p (h wp t) -> p h wp t", h=8, wp=32, t=2)
            gm = sm.tile([128, 8, 32], F32, tag="gm")
            nc.vector.tensor_tensor(out=gm, in0=gr[:, :, :, 0], in1=gr[:, :, :, 1],
                                    op=ALU.max)
            gm2 = gm.rearrange("p (hp s) wp -> p hp s wp", s=2)
            nc.vector.tensor_tensor(
                out=g_pool[:, ms].rearrange("p (hp wp) -> p hp wp", hp=4),
                in0=gm2[:, :, 0, :], in1=gm2[:, :, 1, :], op=ALU.max)
            nc.vector.tensor_scalar_add(out=g_pool[:, ms], in0=g_pool[:, ms],
                                        scalar1=bg_sb)
            tp = ps_rb.tile([128, 128], F32, tag="rb")
            nc.tensor.transpose(tp, g_pool[:, ms], ident)
            nc.scalar.copy(out=gT[:, it, :], in_=tp)
            # x16 copies interleaved (vector)
            for ch in range(CCH):
                nc.vector.tensor_copy(out=x16[ch][:, sl], in_=x_sb[ch][:, sl])

        # ---- attention + W-conv, per n-tile ----
        wy16 = [persist.tile([128, N], F16, tag=f"wy{ch}", name=f"wy16_{ch}")
                for ch in range(CCH)]
        s1p = persist.tile([128, CCH, NTILES], F32, tag="s1p")
        s2p = persist.tile([128, CCH, NTILES], F32, tag="s2p")

        for it in range(NTILES):
            sl = slice(it * NT, (it + 1) * NT)
            yps = ps_y.tile([128, NT], F32, tag="yps")
            sps = ps_s.tile([1, NT], F32, tag="sps")
            for mc in range(MCH):
                ms = slice(mc * 128, (mc + 1) * 128)
                fps = ps_ft.tile([128, NT], F32, tag="ft")
                nc.tensor.matmul(fps, lhsT=phi_pool[:, ms], rhs=th[:, sl],
                                 start=True, stop=True)
                ef = efp.tile([128, NT], F32R, tag="ef")
                nc.scalar.activation(out=ef, in_=fps, func=AF.Exp)
                nc.tensor.matmul(yps, lhsT=gT[:, mc, :], rhs=ef,
                                 start=(mc == 0), stop=(mc == MCH - 1))
                nc.tensor.matmul(sps, lhsT=ones_k, rhs=ef,
                                 start=(mc == 0), stop=(mc == MCH - 1))

            # reciprocal + partition-broadcast via K=1 matmul
            r32 = small.tile([1, NT], F32, tag="r32")
            nc.vector.reciprocal(out=r32, in_=sps)
            rr = small.tile([1, NT], F32R, tag="rr")
            nc.gpsimd.tensor_copy(out=rr, in_=r32)
            rbps = ps_rb.tile([128, NT], F32, tag="rb")
            nc.tensor.matmul(rbps, lhsT=ones_p, rhs=rr, start=True, stop=True)
            rb_sb = sm.tile([128, NT], F32, tag="rbsb")
            nc.vector.tensor_copy(out=rb_sb, in_=rbps)

            # y -> SBUF (f32r) for the W conv
            y_sb = sm.tile([128, NT], F32R, tag="ysb")
            nc.vector.tensor_copy(out=y_sb, in_=yps)

            for ch in range(CCH):
                wps = ps_cv.tile([128, NT], F32, tag="cv")
                nc.tensor.matmul(wps, lhsT=ww_sb[:, ch, :], rhs=y_sb,
                                 start=True, stop=True)
                # wy_n = wps * rb  (fp16), fused BN s1 accumulation
                nc.vector.scalar_tensor_tensor(
                    out=wy16[ch][:, sl], in0=wps, scalar=1.0, in1=rb_sb,
                    op0=ALU.mult, op1=ALU.mult,
                    accum_out=s1p[:, ch, it:it + 1])
                # BN s2: square on gpsimd, reduce on vector
                sq = sm.tile([128, NT], F32, tag="sq")
                nc.gpsimd.tensor_tensor(out=sq, in0=wy16[ch][:, sl],
                                        in1=wy16[ch][:, sl], op=ALU.mult)
                nc.vector.tensor_reduce(out=s2p[:, ch, it:it + 1], in_=sq,
                                        axis=AX.X, op=ALU.add)

        # ---- combine partials, AllReduce, finalize ----
        stats_sb = small.tile([128, 2 * CCH], F32, tag="stats")
        for ch in range(CCH):
            nc.vector.tensor_reduce(out=stats_sb[:, 2 * ch:2 * ch + 1],
                                    in_=s1p[:, ch, :], axis=AX.X, op=ALU.add)
            nc.vector.tensor_reduce(out=stats_sb[:, 2 * ch + 1:2 * ch + 2],
                                    in_=s2p[:, ch, :], axis=AX.X, op=ALU.add)
        nc.sync.dma_start(out=stats_in[:, :], in_=stats_sb)
        nc.gpsimd.collective_compute(
            "AllReduce", ALU.add, replica_groups=[list(range(B))],
            ins=[stats_in[:, :]], outs=[stats_out[:, :]])
        stats_g = small.tile([128, 2 * CCH], F32, tag="statsg")
        nc.sync.dma_start(out=stats_g, in_=stats_out[:, :])

        out_sb = small.tile([128, CCH], F32, tag="outsb")
        for ch in range(CCH):
            mean = small.tile([128, 1], F32, tag="fin")
            e2 = small.tile([128, 1], F32, tag="fin")
            var = small.tile([128, 1], F32, tag="fin")
            nc.vector.tensor_scalar_mul(out=mean, in0=stats_g[:, 2 * ch:2 * ch + 1],
                                        scalar1=INV_CNT)
            nc.vector.tensor_scalar_mul(out=e2, in0=stats_g[:, 2 * ch + 1:2 * ch + 2],
                                        scalar1=INV_CNT)
            m2 = small.tile([128, 1], F32, tag="fin")
            nc.scalar.square(out=m2, in_=mean)
            nc.vector.tensor_tensor(out=var, in0=e2, in1=m2, op=ALU.subtract)
            # rstd = exp(-0.5 * ln(var + eps)) -- stays in the exp/ln ACT table
            lnv = small.tile([128, 1], F32, tag="fin")
            nc.scalar.activation(out=lnv, in_=var, func=AF.Ln, bias=eps_sb,
                                 scale=1.0)
            rstd = small.tile([128, 1], F32, tag="fin")
            nc.scalar.activation(out=rstd, in_=lnv, func=AF.Exp, scale=-0.5)
            scale = small.tile([128, 1], F32, tag="fin")
            nc.vector.tensor_tensor(out=scale, in0=rstd,
                                    in1=gamma_sb[:, ch:ch + 1], op=ALU.mult)
            negshift = small.tile([128, 1], F32, tag="fin")
            nc.vector.scalar_tensor_tensor(
                out=negshift, in0=mean, scalar=scale, in1=beta_sb[:, ch:ch + 1],
                op0=ALU.mult, op1=ALU.subtract)
            # z = wy16*scale + x16 ; out = max_n z - negshift
            z = sm.tile([128, N], F16, tag=f"z{ch}")
            nc.vector.scalar_tensor_tensor(out=z, in0=wy16[ch][:, :], scalar=scale,
                                           in1=x16[ch][:, :], op0=ALU.mult,
                                           op1=ALU.add)
            mx = small.tile([128, 1], F32, tag="fin")
            nc.vector.tensor_reduce(out=mx, in_=z, axis=AX.X, op=ALU.max)
            nc.vector.tensor_tensor(out=out_sb[:, ch:ch + 1], in0=mx,
                                    in1=negshift, op=ALU.subtract)
        for ch in range(CCH):
            nc.sync.dma_start(out=out_d[ch, :].rearrange("(p one) -> p one", one=1),
                              in_=out_sb[:, ch:ch + 1])

    nc.compile()
    return nc


_LAST = {}


def kernel(**inputs):
    x = np.ascontiguousarray(inputs["x"], dtype=np.float32)      # (8, 256, 64, 64)
    Wg = np.asarray(inputs["Wg"], dtype=np.float32)
    bg = np.asarray(inputs["bg"], dtype=np.float32)
    Wt = np.asarray(inputs["Wt"], dtype=np.float32)
    bt = np.asarray(inputs["bt"], dtype=np.float32)
    Wp = np.asarray(inputs["Wp"], dtype=np.float32)
    bp = np.asarray(inputs["bp"], dtype=np.float32)
    Ww = np.asarray(inputs["Ww"], dtype=np.float32)
    bw = np.asarray(inputs["bw"], dtype=np.float32)
    gamma = np.asarray(inputs["gamma"], dtype=np.float32)
    beta = np.asarray(inputs["beta"], dtype=np.float32)

    if "nc" not in _CACHE:
        _CACHE["nc"] = _build()
    nc = _CACHE["nc"]

    shared = {
        "WtT": np.ascontiguousarray(Wt.T),
        "WpT": np.ascontiguousarray(Wp.T),
        "WgT": np.ascontiguousarray(Wg.T),
        "WwT": np.ascontiguousarray(Ww.T),
        "bt": np.ascontiguousarray(bt.reshape(CI, 1)),
        "bp": np.ascontiguousarray(bp.reshape(CI, 1)),
        "bg": np.ascontiguousarray(bg.reshape(CI, 1)),
        "gamma": np.ascontiguousarray(gamma.reshape(CCH, 128).T),
        "beta": np.ascontiguousarray(beta.reshape(CCH, 128).T),
        "ones_k": np.ones((128, 1), dtype=np.float32),
        "ones_p": np.ones((1, 128), dtype=np.float32),
    }
    in_maps = [dict(shared, x=np.ascontiguousarray(x[b].reshape(C, N)))
               for b in range(B)]
    import os
    trace = bool(int(os.environ.get("KERNEL_TRACE", "0")))
    res = run_bass_kernel_spmd(nc, in_maps, core_ids=list(range(B)), trace=trace)
    _LAST["res"] = res
    out = np.stack([np.asarray(res.results[b]["out"]).reshape(C) for b in range(B)])
    return out.reshape(B, C, 1, 1).astype(np.float32)


if __name__ == "__main__":
    pass


# revision 8
# speedup vs baseline: 1.7295x; 1.1426x over previous
"""Trainium2 Bass kernel for the non-local attention block (nn_CPP_80676665688885).

Sharding: pure data-parallel over batch — 1 sample per NeuronCore (B=8, 8 cores).
BatchNorm batch-statistics are combined with a tiny (2 KB) AllReduce.

All matmuls run in float32r (single-pass, 1 col/cycle like bf16, ~13-bit
mantissa) — no hi/lo splitting needed anywhere. exp() output stays in f32r.
Softmax normalization (1/s) is deferred past the W-conv: (Ww@y)/s == Ww@(y/s),
so the per-tile critical chain is fT -> exp -> y -> wconv with the reciprocal
path (s -> 1/s -> partition-broadcast via K=1 matmul) running in parallel.

Per-core algorithm (sample x: (C=256, N=4096), N = 64x64 spatial):
  theta = Wt@x + bt                       phi,g = maxpool2(conv) + bias post-pool
  per n-tile (512 cols), per m-chunk (128):
    fT   = phi^T @ theta   (f32r)  ; ef = exp(fT) (f32r, ScalarE)
    y   += gT^T @ ef ; s += ones^T @ ef
  r = 1/s ; rb = broadcast(r) via ones(1,128)^T @ r matmul
  wy_n = (Ww @ y) * rb  -> fp16, with fused Σ (BN s1); Σwy² via gpsimd+vector
  stats AllReduce over 8 cores ; scale = gamma*rsqrt(var+eps) (rsqrt = exp(-½ln))
  out[c] = max_n(wy_n*scale + x) + (beta - mean*scale)
"""

import numpy as np
from contextlib import ExitStack

import concourse.bass as bass
import concourse.bacc as bacc
import concourse.tile as tile
from concourse import mybir
from concourse.bass_utils import run_bass_kernel_spmd

F32 = mybir.dt.float32
F32R = mybir.dt.float32r
F16 = mybir.dt.float16
BF16 = mybir.dt.bfloat16
AF = mybir.ActivationFunctionType
ALU = mybir.AluOpType
AX = mybir.AxisListType

B = 8
C = 256
CI = 128
N = 4096          # 64*64
M = 1024          # 32*32 after 2x2 maxpool
NT = 512          # n-tile (PSUM bank width in fp32)
NTILES = N // NT  # 8
MCH = M // 128    # 8 m-chunks
CCH = C // 128    # 2 channel chunks
EPS = 1e-5
INV_CNT = 1.0 / (B * N)

_CACHE = {}


def _build():
    nc = bacc.Bacc("TRN2", num_devices=B)

    x_d = nc.declare_dram_parameter("x", [C, N], F32R, False)
    w_d = {}
    for nm in ("t", "p", "g"):
        w_d[nm] = nc.declare_dram_parameter(f"W{nm}T", [C, CI], F32R, False)
    wwT_d = nc.declare_dram_parameter("WwT", [CI, C], BF16, False)
    bt_d = nc.declare_dram_parameter("bt", [CI, 1], F32, False)
    bp_d = nc.declare_dram_parameter("bp", [CI, 1], F32, False)
    bg_d = nc.declare_dram_parameter("bg", [CI, 1], F32, False)
    gamma_d = nc.declare_dram_parameter("gamma", [128, CCH], F32, False)
    beta_d = nc.declare_dram_parameter("beta", [128, CCH], F32, False)
    ones_k_d = nc.declare_dram_parameter("ones_k", [128, 1], BF16, False)
    ones_p_d = nc.declare_dram_parameter("ones_p", [1, 128], F32R, False)
    out_d = nc.declare_dram_parameter("out", [CCH, 128], F32, True)

    ident_d = nc.inline_tensor(np.eye(128, dtype=np.float32), name="ident")

    warm_in = nc.dram_tensor("warm_in", [1, 8], F32)
    warm_out = nc.dram_tensor("warm_out", [1, 8], F32, addr_space="Shared")
    stats_in = nc.dram_tensor("stats_in", [128, 2 * CCH], F32)
    stats_out = nc.dram_tensor("stats_out", [128, 2 * CCH], F32,
                               addr_space="Shared")

    with ExitStack() as ctx:
        tc = ctx.enter_context(tile.TileContext(nc))
        consts = ctx.enter_context(tc.tile_pool(name="consts", bufs=1))
        persist = ctx.enter_context(tc.tile_pool(name="persist", bufs=1))
        efp = ctx.enter_context(tc.tile_pool(name="efp", bufs=4))
        sm = ctx.enter_context(tc.tile_pool(name="sm", bufs=2))
        small = ctx.enter_context(tc.tile_pool(name="small", bufs=4))
        # PSUM: 8 banks total
        ps_cv = ctx.enter_context(tc.tile_pool(name="ps_cv", bufs=2, space="PSUM"))
        ps_ft = ctx.enter_context(tc.tile_pool(name="ps_ft", bufs=2, space="PSUM"))
        ps_y = ctx.enter_context(tc.tile_pool(name="ps_y", bufs=2, space="PSUM"))
        ps_s = ctx.enter_context(tc.tile_pool(name="ps_s", bufs=1, space="PSUM"))
        ps_rb = ctx.enter_context(tc.tile_pool(name="ps_rb", bufs=1, space="PSUM"))

        # ---- constants / weights into SBUF ----
        ident = consts.tile([128, 128], F32)
        nc.sync.dma_start(out=ident, in_=ident_d[:, :])
        ones_k = consts.tile([128, 1], BF16)
        nc.sync.dma_start(out=ones_k, in_=ones_k_d[:, :])
        ones_p = consts.tile([1, 128], F32R)
        nc.sync.dma_start(out=ones_p, in_=ones_p_d[:, :])
        eps_sb = consts.tile([128, 1], F32)
        nc.vector.memset(eps_sb, EPS)

        w_sb = {}
        for nm in ("t", "p", "g"):
            w_sb[nm] = consts.tile([128, CCH, CI], F32R, name=f"w_{nm}")
            for ch in range(CCH):
                nc.sync.dma_start(out=w_sb[nm][:, ch, :],
                                  in_=w_d[nm][ch * 128:(ch + 1) * 128, :])
        ww_sb = consts.tile([128, CCH, 128], BF16)
        for ch in range(CCH):
            nc.sync.dma_start(out=ww_sb[:, ch, :],
                              in_=wwT_d[:, ch * 128:(ch + 1) * 128])
        bt_sb = consts.tile([128, 1], F32)
        bp_sb = consts.tile([128, 1], F32)
        bg_sb = consts.tile([128, 1], F32)
        nc.sync.dma_start(out=bt_sb, in_=bt_d[:, :])
        nc.sync.dma_start(out=bp_sb, in_=bp_d[:, :])
        nc.sync.dma_start(out=bg_sb, in_=bg_d[:, :])
        gamma_sb = consts.tile([128, CCH], F32)
        beta_sb = consts.tile([128, CCH], F32)
        nc.sync.dma_start(out=gamma_sb, in_=gamma_d[:, :])
        nc.sync.dma_start(out=beta_sb, in_=beta_d[:, :])

        # warm up the collective path early (overlaps with compute)
        warm_sb = small.tile([1, 8], F32, tag="warm")
        nc.vector.memset(warm_sb, 1.0)
        nc.sync.dma_start(out=warm_in[:, :], in_=warm_sb)
        nc.gpsimd.collective_compute(
            "AllReduce", ALU.add, replica_groups=[list(range(B))],
            ins=[warm_in[:, :]], outs=[warm_out[:, :]])

        # ---- x into SBUF (chunked DMA so convs can start early) ----
        x_sb = [persist.tile([128, N], F32R, tag=f"x{ch}", name=f"x_sb{ch}")
                for ch in range(CCH)]
        QN = N // 4
        for q in range(4):
            qs = slice(q * QN, (q + 1) * QN)
            for ch in range(CCH):
                nc.sync.dma_start(out=x_sb[ch][:, qs],
                                  in_=x_d[ch * 128:(ch + 1) * 128, qs])

        # fp16 copy of x for the finale (vector, overlaps conv matmuls)
        x16 = [persist.tile([128, N], F16, tag=f"x16_{ch}", name=f"x16_{ch}")
               for ch in range(CCH)]

        # ---- projections ----
        th = persist.tile([128, N], F32R, tag="th")
        phi_pool = persist.tile([128, M], F32R, tag="phip")
        g_pool = persist.tile([128, M], F32, tag="gp")
        gT = persist.tile([128, MCH, CI], BF16, tag="gT")

        def conv_mms(ps, nm, sl):
            for ch in range(CCH):
                nc.tensor.matmul(ps, lhsT=w_sb[nm][:, ch, :], rhs=x_sb[ch][:, sl],
                                 start=(ch == 0), stop=(ch == CCH - 1))

        for it in range(NTILES):
            sl = slice(it * NT, (it + 1) * NT)
            ms = slice(it * 128, (it + 1) * 128)
            # theta
            ps = ps_cv.tile([128, NT], F32, tag="cv")
            conv_mms(ps, "t", sl)
            nc.vector.tensor_scalar_add(out=th[:, sl], in0=ps, scalar1=bt_sb)
            # phi conv -> copy -> 2x2 maxpool -> +bias
            psp = ps_cv.tile([128, NT], F32, tag="cv")
            conv_mms(psp, "p", sl)
            pcp = sm.tile([128, NT], F32, tag="pcp")
            nc.scalar.copy(out=pcp, in_=psp)
            pr = pcp.rearrange("p (h wp t) -> p h wp t", h=8, wp=32, t=2)
            pm = sm.tile([128, 8, 32], F32, tag="pm")
            nc.vector.tensor_tensor(out=pm, in0=pr[:, :, :, 0], in1=pr[:, :, :, 1],
                                    op=ALU.max)
            pm2 = pm.rearrange("p (hp s) wp -> p hp s wp", s=2)
            pp = sm.tile([128, 128], F32, tag="pp")
            nc.vector.tensor_tensor(
                out=pp.rearrange("p (hp wp) -> p hp wp", hp=4),
                in0=pm2[:, :, 0, :], in1=pm2[:, :, 1, :], op=ALU.max)
            nc.vector.tensor_scalar_add(out=phi_pool[:, ms], in0=pp,
                                        scalar1=bp_sb)
            # g conv -> copy -> maxpool -> +bias -> transpose
            psg = ps_cv.tile([128, NT], F32, tag="cv")
            conv_mms(psg, "g", sl)
            gcp = sm.tile([128, NT], F32, tag="gcp")
            nc.vector.tensor_copy(out=gcp, in_=psg)
            gr = gcp.rearrange("p (h wp t) -> p h wp t", h=8, wp=32, t=2)
            gm = sm.tile([128, 8, 32], F32, tag="gm")
            nc.vector.tensor_tensor(out=gm, in0=gr[:, :, :, 0], in1=gr[:, :, :, 1],
                                    op=ALU.max)
            gm2 = gm.rearrange("p (hp s) wp -> p hp s wp", s=2)
            nc.vector.tensor_tensor(
                out=g_pool[:, ms].rearrange("p (hp wp) -> p hp wp", hp=4),
                in0=gm2[:, :, 0, :], in1=gm2[:, :, 1, :], op=ALU.max)
            nc.vector.tensor_scalar_add(out=g_pool[:, ms], in0=g_pool[:, ms],
                                        scalar1=bg_sb)
            tp = ps_rb.tile([128, 128], F32, tag="rb")
            nc.tensor.transpose(tp, g_pool[:, ms], ident)
            nc.scalar.copy(out=gT[:, it, :], in_=tp)
            # x16 copies interleaved (vector)
            for ch in range(CCH):
                nc.vector.tensor_copy(out=x16[ch][:, sl], in_=x_sb[ch][:, sl])

        # ---- attention + W-conv, per n-tile ----
        wy16 = [persist.tile([128, N], F16, tag=f"wy{ch}", name=f"wy16_{ch}")
                for ch in range(CCH)]
        s1p = persist.tile([128, CCH, NTILES], F32, tag="s1p")
        s2p = persist.tile([128, CCH, NTILES], F32, tag="s2p")

        for it in range(NTILES):
            sl = slice(it * NT, (it + 1) * NT)
            yps = ps_y.tile([128, NT], F32, tag="yps")
            sps = ps_s.tile([1, NT], F32, tag="sps")
            for mc in range(MCH):
                ms = slice(mc * 128, (mc + 1) * 128)
                fps = ps_ft.tile([128, NT], F32, tag="ft")
                nc.tensor.matmul(fps, lhsT=phi_pool[:, ms], rhs=th[:, sl],
                                 start=True, stop=True)
                ef = efp.tile([128, NT], BF16, tag="ef")
                nc.scalar.activation(out=ef, in_=fps, func=AF.Exp)
                nc.tensor.matmul(yps, lhsT=gT[:, mc, :], rhs=ef,
                                 start=(mc == 0), stop=(mc == MCH - 1))
                nc.tensor.matmul(sps, lhsT=ones_k, rhs=ef,
                                 start=(mc == 0), stop=(mc == MCH - 1))

            # reciprocal + partition-broadcast via K=1 matmul
            r32 = small.tile([1, NT], F32, tag="r32")
            nc.vector.reciprocal_approx_fast(out=r32, in_=sps)
            rr = small.tile([1, NT], F32R, tag="rr")
            nc.gpsimd.tensor_copy(out=rr, in_=r32)
            rbps = ps_rb.tile([128, NT], F32, tag="rb")
            nc.tensor.matmul(rbps, lhsT=ones_p, rhs=rr, start=True, stop=True)
            rb_sb = sm.tile([128, NT], F32, tag="rbsb")
            nc.vector.tensor_copy(out=rb_sb, in_=rbps)

            # y -> SBUF (f32r) for the W conv
            y_sb = sm.tile([128, NT], BF16, tag="ysb")
            nc.vector.tensor_copy(out=y_sb, in_=yps)

            for ch in range(CCH):
                wps = ps_cv.tile([128, NT], F32, tag="cv")
                nc.tensor.matmul(wps, lhsT=ww_sb[:, ch, :], rhs=y_sb,
                                 start=True, stop=True)
                # wy_n = wps * rb  (fp16), fused BN s1 accumulation
                nc.vector.scalar_tensor_tensor(
                    out=wy16[ch][:, sl], in0=wps, scalar=1.0, in1=rb_sb,
                    op0=ALU.mult, op1=ALU.mult,
                    accum_out=s1p[:, ch, it:it + 1])
                # BN s2: square on gpsimd, reduce on vector
                sq = sm.tile([128, NT], F32, tag="sq")
                nc.vector.tensor_tensor(out=sq, in0=wy16[ch][:, sl],
                                        in1=wy16[ch][:, sl], op=ALU.mult)
                nc.vector.tensor_reduce(out=s2p[:, ch, it:it + 1], in_=sq,
                                        axis=AX.X, op=ALU.add)

        # ---- combine partials, AllReduce, finalize ----
        stats_sb = small.tile([128, 2 * CCH], F32, tag="stats")
        for ch in range(CCH):
            nc.vector.tensor_reduce(out=stats_sb[:, 2 * ch:2 * ch + 1],
                                    in_=s1p[:, ch, :], axis=AX.X, op=ALU.add)
            nc.vector.tensor_reduce(out=stats_sb[:, 2 * ch + 1:2 * ch + 2],
                                    in_=s2p[:, ch, :], axis=AX.X, op=ALU.add)
        nc.sync.dma_start(out=stats_in[:, :], in_=stats_sb)
        nc.gpsimd.collective_compute(
            "AllReduce", ALU.add, replica_groups=[list(range(B))],
            ins=[stats_in[:, :]], outs=[stats_out[:, :]])
        stats_g = small.tile([128, 2 * CCH], F32, tag="statsg")
        nc.sync.dma_start(out=stats_g, in_=stats_out[:, :])

        out_sb = small.tile([128, CCH], F32, tag="outsb")
        for ch in range(CCH):
            mean = small.tile([128, 1], F32, tag="fin")
            e2 = small.tile([128, 1], F32, tag="fin")
            var = small.tile([128, 1], F32, tag="fin")
            nc.vector.tensor_scalar_mul(out=mean, in0=stats_g[:, 2 * ch:2 * ch + 1],
                                        scalar1=INV_CNT)
            nc.vector.tensor_scalar_mul(out=e2, in0=stats_g[:, 2 * ch + 1:2 * ch + 2],
                                        scalar1=INV_CNT)
            m2 = small.tile([128, 1], F32, tag="fin")
            nc.scalar.square(out=m2, in_=mean)
            nc.vector.tensor_tensor(out=var, in0=e2, in1=m2, op=ALU.subtract)
            # rstd = exp(-0.5 * ln(var + eps)) -- stays in the exp/ln ACT table
            lnv = small.tile([128, 1], F32, tag="fin")
            nc.scalar.activation(out=lnv, in_=var, func=AF.Ln, bias=eps_sb,
                                 scale=1.0)
            rstd = small.tile([128, 1], F32, tag="fin")
            nc.scalar.activation(out=rstd, in_=lnv, func=AF.Exp, scale=-0.5)
            scale = small.tile([128, 1], F32, tag="fin")
            nc.vector.tensor_tensor(out=scale, in0=rstd,
                                    in1=gamma_sb[:, ch:ch + 1], op=ALU.mult)
            negshift = small.tile([128, 1], F32, tag="fin")
            nc.vector.scalar_tensor_tensor(
                out=negshift, in0=mean, scalar=scale, in1=beta_sb[:, ch:ch + 1],
                op0=ALU.mult, op1=ALU.subtract)
            # z = wy16*scale + x16 ; out = max_n z - negshift
            z = sm.tile([128, N], F16, tag=f"z{ch}")
            nc.vector.scalar_tensor_tensor(out=z, in0=wy16[ch][:, :], scalar=scale,
                                           in1=x16[ch][:, :], op0=ALU.mult,
                                           op1=ALU.add)
            mx = small.tile([128, 1], F32, tag="fin")
            nc.vector.tensor_reduce(out=mx, in_=z, axis=AX.X, op=ALU.max)
            nc.vector.tensor_tensor(out=out_sb[:, ch:ch + 1], in0=mx,
                                    in1=negshift, op=ALU.subtract)
        for ch in range(CCH):
            nc.sync.dma_start(out=out_d[ch, :].rearrange("(p one) -> p one", one=1),
                              in_=out_sb[:, ch:ch + 1])

    nc.compile()
    return nc


_LAST = {}


def _to_bf16(a):
    try:
        import ml_dtypes
        return np.ascontiguousarray(a.astype(ml_dtypes.bfloat16))
    except ImportError:
        import jax.numpy as jnp
        return np.ascontiguousarray(np.asarray(jnp.asarray(a, dtype=jnp.bfloat16)))


def kernel(**inputs):
    x = np.ascontiguousarray(inputs["x"], dtype=np.float32)      # (8, 256, 64, 64)
    Wg = np.asarray(inputs["Wg"], dtype=np.float32)
    bg = np.asarray(inputs["bg"], dtype=np.float32)
    Wt = np.asarray(inputs["Wt"], dtype=np.float32)
    bt = np.asarray(inputs["bt"], dtype=np.float32)
    Wp = np.asarray(inputs["Wp"], dtype=np.float32)
    bp = np.asarray(inputs["bp"], dtype=np.float32)
    Ww = np.asarray(inputs["Ww"], dtype=np.float32)
    bw = np.asarray(inputs["bw"], dtype=np.float32)
    gamma = np.asarray(inputs["gamma"], dtype=np.float32)
    beta = np.asarray(inputs["beta"], dtype=np.float32)

    if "nc" not in _CACHE:
        _CACHE["nc"] = _build()
    nc = _CACHE["nc"]

    shared = {
        "WtT": np.ascontiguousarray(Wt.T),
        "WpT": np.ascontiguousarray(Wp.T),
        "WgT": np.ascontiguousarray(Wg.T),
        "WwT": _to_bf16(np.ascontiguousarray(Ww.T)),
        "bt": np.ascontiguousarray(bt.reshape(CI, 1)),
        "bp": np.ascontiguousarray(bp.reshape(CI, 1)),
        "bg": np.ascontiguousarray(bg.reshape(CI, 1)),
        "gamma": np.ascontiguousarray(gamma.reshape(CCH, 128).T),
        "beta": np.ascontiguousarray(beta.reshape(CCH, 128).T),
        "ones_k": _to_bf16(np.ones((128, 1), dtype=np.float32)),
        "ones_p": np.ones((1, 128), dtype=np.float32),
    }
    in_maps = [dict(shared, x=np.ascontiguousarray(x[b].reshape(C, N)))
               for b in range(B)]
    import os
    trace = bool(int(os.environ.get("KERNEL_TRACE", "0")))
    res = run_bass_kernel_spmd(nc, in_maps, core_ids=list(range(B)), trace=trace)
    _LAST["res"] = res
    out = np.stack([np.asarray(res.results[b]["out"]).reshape(C) for b in range(B)])
    return out.reshape(B, C, 1, 1).astype(np.float32)


if __name__ == "__main__":
    pass


# revision 11
# speedup vs baseline: 1.8106x; 1.0469x over previous
"""Trainium2 Bass kernel for the non-local attention block (nn_CPP_80676665688885).

Sharding: pure data-parallel over batch — 1 sample per NeuronCore (B=8, 8 cores).
BatchNorm batch-statistics are combined with a tiny (2 KB) AllReduce.

All matmuls run in float32r (single-pass, 1 col/cycle like bf16, ~13-bit
mantissa) — no hi/lo splitting needed anywhere. exp() output stays in f32r.
Softmax normalization (1/s) is deferred past the W-conv: (Ww@y)/s == Ww@(y/s),
so the per-tile critical chain is fT -> exp -> y -> wconv with the reciprocal
path (s -> 1/s -> partition-broadcast via K=1 matmul) running in parallel.

Per-core algorithm (sample x: (C=256, N=4096), N = 64x64 spatial):
  theta = Wt@x + bt                       phi,g = maxpool2(conv) + bias post-pool
  per n-tile (512 cols), per m-chunk (128):
    fT   = phi^T @ theta   (f32r)  ; ef = exp(fT) (f32r, ScalarE)
    y   += gT^T @ ef ; s += ones^T @ ef
  r = 1/s ; rb = broadcast(r) via ones(1,128)^T @ r matmul
  wy_n = (Ww @ y) * rb  -> fp16, with fused Σ (BN s1); Σwy² via gpsimd+vector
  stats AllReduce over 8 cores ; scale = gamma*rsqrt(var+eps) (rsqrt = exp(-½ln))
  out[c] = max_n(wy_n*scale + x) + (beta - mean*scale)
"""

import numpy as np
from contextlib import ExitStack

import concourse.bass as bass
import concourse.bacc as bacc
import concourse.tile as tile
from concourse import mybir
from concourse.bass_utils import run_bass_kernel_spmd

F32 = mybir.dt.float32
F32R = mybir.dt.float32r
F16 = mybir.dt.float16
BF16 = mybir.dt.bfloat16
AF = mybir.ActivationFunctionType
ALU = mybir.AluOpType
AX = mybir.AxisListType

B = 8
C = 256
CI = 128
N = 4096          # 64*64
M = 1024          # 32*32 after 2x2 maxpool
NT = 512          # n-tile (PSUM bank width in fp32)
NTILES = N // NT  # 8
MCH = M // 128    # 8 m-chunks
CCH = C // 128    # 2 channel chunks
EPS = 1e-5
INV_CNT = 1.0 / (B * N)

_CACHE = {}


def _build():
    nc = bacc.Bacc("TRN2", num_devices=B)

    x_d = nc.declare_dram_parameter("x", [C, N], F32R, False)
    w_d = {}
    for nm in ("t", "p", "g"):
        w_d[nm] = nc.declare_dram_parameter(f"W{nm}T", [C, CI], F32R, False)
    wwT_d = nc.declare_dram_parameter("WwT", [CI, C], BF16, False)
    smalls_d = nc.declare_dram_parameter("smalls", [128, 7], F32, False)
    ones_k_d = nc.declare_dram_parameter("ones_k", [128, 1], BF16, False)
    ones_p_d = nc.declare_dram_parameter("ones_p", [1, 128], BF16, False)
    out_d = nc.declare_dram_parameter("out", [CCH, 128], F32, True)

    ident_d = nc.inline_tensor(np.eye(128, dtype=np.float32), name="ident")

    warm_in = nc.dram_tensor("warm_in", [1, 8], F32)
    warm_out = nc.dram_tensor("warm_out", [1, 8], F32, addr_space="Shared")
    stats_in = nc.dram_tensor("stats_in", [128, 2 * CCH], F32)
    stats_out = nc.dram_tensor("stats_out", [128, 2 * CCH], F32,
                               addr_space="Shared")

    with ExitStack() as ctx:
        tc = ctx.enter_context(tile.TileContext(nc))
        consts = ctx.enter_context(tc.tile_pool(name="consts", bufs=1))
        persist = ctx.enter_context(tc.tile_pool(name="persist", bufs=1))
        efp = ctx.enter_context(tc.tile_pool(name="efp", bufs=4))
        sm = ctx.enter_context(tc.tile_pool(name="sm", bufs=2))
        small = ctx.enter_context(tc.tile_pool(name="small", bufs=4))
        # PSUM: 8 banks total
        ps_ft = ctx.enter_context(tc.tile_pool(name="ps_ft", bufs=3, space="PSUM"))
        ps_y = ctx.enter_context(tc.tile_pool(name="ps_y", bufs=2, space="PSUM"))
        ps_s = ctx.enter_context(tc.tile_pool(name="ps_s", bufs=1, space="PSUM"))
        ps_mix = ctx.enter_context(tc.tile_pool(name="ps_mix", bufs=2, space="PSUM"))

        # ---- x DMAs first (sync queue), weights on scalar queue, consts on vector ----
        x_sb = [persist.tile([128, N], F32R, tag=f"x{ch}", name=f"x_sb{ch}")
                for ch in range(CCH)]
        QN = N // 4
        for q in range(4):
            qs = slice(q * QN, (q + 1) * QN)
            for ch in range(CCH):
                nc.sync.dma_start(out=x_sb[ch][:, qs],
                                  in_=x_d[ch * 128:(ch + 1) * 128, qs])

        w_sb = {}
        for nm in ("t", "p", "g"):
            w_sb[nm] = consts.tile([128, CCH, CI], F32R, name=f"w_{nm}")
            nc.scalar.dma_start(
                out=w_sb[nm][:, :, :],
                in_=w_d[nm].rearrange("(c2 p) ci -> p c2 ci", p=128))
        ww_sb = consts.tile([128, CCH, 128], BF16)
        nc.scalar.dma_start(out=ww_sb[:, :, :],
                            in_=wwT_d.rearrange("p (c2 k) -> p c2 k", c2=CCH))
        smalls = consts.tile([128, 7], F32)
        nc.scalar.dma_start(out=smalls, in_=smalls_d[:, :])
        bt_sb = smalls[:, 0:1]
        bp_sb = smalls[:, 1:2]
        bg_sb = smalls[:, 2:3]
        gamma_sb = smalls[:, 3:5]
        beta_sb = smalls[:, 5:7]

        ident = consts.tile([128, 128], F32)
        nc.gpsimd.dma_start(out=ident, in_=ident_d[:, :])
        ones_k = consts.tile([128, 1], BF16)
        nc.gpsimd.dma_start(out=ones_k, in_=ones_k_d[:, :])
        ones_p = consts.tile([1, 128], BF16)
        nc.gpsimd.dma_start(out=ones_p, in_=ones_p_d[:, :])
        eps_sb = consts.tile([128, 1], F32)
        nc.vector.memset(eps_sb, EPS)

        # warm up the collective path early (overlaps with compute)
        warm_sb = small.tile([1, 8], F32, tag="warm")
        nc.vector.memset(warm_sb, 1.0)
        nc.gpsimd.dma_start(out=warm_in[:, :], in_=warm_sb)
        nc.gpsimd.collective_compute(
            "AllReduce", ALU.add, replica_groups=[list(range(B))],
            ins=[warm_in[:, :]], outs=[warm_out[:, :]])

        # fp16 copy of x for the finale (vector, overlaps conv matmuls)
        x16 = [persist.tile([128, N], F16, tag=f"x16_{ch}", name=f"x16_{ch}")
               for ch in range(CCH)]

        # ---- projections ----
        th = persist.tile([128, N], F32R, tag="th")
        phi_pool = persist.tile([128, M], F32R, tag="phip")
        g_pool = persist.tile([128, M], F32, tag="gp")
        gT = persist.tile([128, MCH, CI], BF16, tag="gT")

        def conv_mms(ps, nm, sl):
            for ch in range(CCH):
                nc.tensor.matmul(ps, lhsT=w_sb[nm][:, ch, :], rhs=x_sb[ch][:, sl],
                                 start=(ch == 0), stop=(ch == CCH - 1))

        for it in range(NTILES):
            sl = slice(it * NT, (it + 1) * NT)
            ms = slice(it * 128, (it + 1) * 128)
            # theta
            ps = ps_ft.tile([128, NT], F32, tag="ft")
            conv_mms(ps, "t", sl)
            nc.vector.tensor_scalar_add(out=th[:, sl], in0=ps, scalar1=bt_sb)
            # phi conv -> copy -> 2x2 maxpool -> +bias
            psp = ps_ft.tile([128, NT], F32, tag="ft")
            conv_mms(psp, "p", sl)
            pcp = sm.tile([128, NT], F32, tag="pcp")
            nc.scalar.copy(out=pcp, in_=psp)
            pr = pcp.rearrange("p (h wp t) -> p h wp t", h=8, wp=32, t=2)
            pm = sm.tile([128, 8, 32], F32, tag="pm")
            nc.vector.tensor_tensor(out=pm, in0=pr[:, :, :, 0], in1=pr[:, :, :, 1],
                                    op=ALU.max)
            pm2 = pm.rearrange("p (hp s) wp -> p hp s wp", s=2)
            pp = sm.tile([128, 128], F32, tag="pp")
            nc.vector.tensor_tensor(
                out=pp.rearrange("p (hp wp) -> p hp wp", hp=4),
                in0=pm2[:, :, 0, :], in1=pm2[:, :, 1, :], op=ALU.max)
            nc.vector.tensor_scalar_add(out=phi_pool[:, ms], in0=pp,
                                        scalar1=bp_sb)
            # g conv -> copy -> maxpool -> +bias -> transpose
            psg = ps_ft.tile([128, NT], F32, tag="ft")
            conv_mms(psg, "g", sl)
            gcp = sm.tile([128, NT], F32, tag="gcp")
            nc.vector.tensor_copy(out=gcp, in_=psg)
            gr = gcp.rearrange("p (h wp t) -> p h wp t", h=8, wp=32, t=2)
            gm = sm.tile([128, 8, 32], F32, tag="gm")
            nc.vector.tensor_tensor(out=gm, in0=gr[:, :, :, 0], in1=gr[:, :, :, 1],
                                    op=ALU.max)
            gm2 = gm.rearrange("p (hp s) wp -> p hp s wp", s=2)
            nc.vector.tensor_tensor(
                out=g_pool[:, ms].rearrange("p (hp wp) -> p hp wp", hp=4),
                in0=gm2[:, :, 0, :], in1=gm2[:, :, 1, :], op=ALU.max)
            nc.vector.tensor_scalar_add(out=g_pool[:, ms], in0=g_pool[:, ms],
                                        scalar1=bg_sb)
            tp = ps_mix.tile([128, 128], F32, tag="mix")
            nc.tensor.transpose(tp, g_pool[:, ms], ident)
            nc.scalar.copy(out=gT[:, it, :], in_=tp)
            # x16 copies interleaved (vector)
            for ch in range(CCH):
                nc.vector.tensor_copy(out=x16[ch][:, sl], in_=x_sb[ch][:, sl])

        # ---- attention + W-conv, per n-tile ----
        wy16 = [persist.tile([128, N], F16, tag=f"wy{ch}", name=f"wy16_{ch}")
                for ch in range(CCH)]
        s1p = persist.tile([128, CCH, NTILES], F32, tag="s1p")
        s2p = persist.tile([128, CCH, NTILES], F32, tag="s2p")

        for it in range(NTILES):
            sl = slice(it * NT, (it + 1) * NT)
            yps = ps_y.tile([128, NT], F32, tag="yps")
            sps = ps_s.tile([1, NT], F32, tag="sps")
            for mc in range(MCH):
                ms = slice(mc * 128, (mc + 1) * 128)
                fps = ps_ft.tile([128, NT], F32, tag="ft")
                nc.tensor.matmul(fps, lhsT=phi_pool[:, ms], rhs=th[:, sl],
                                 start=True, stop=True)
                ef = efp.tile([128, NT], BF16, tag="ef")
                nc.scalar.activation(out=ef, in_=fps, func=AF.Exp)
                nc.tensor.matmul(yps, lhsT=gT[:, mc, :], rhs=ef,
                                 start=(mc == 0), stop=(mc == MCH - 1))
                nc.tensor.matmul(sps, lhsT=ones_k, rhs=ef,
                                 start=(mc == 0), stop=(mc == MCH - 1))

            # reciprocal + partition-broadcast via K=1 matmul
            r32 = small.tile([1, NT], F32, tag="r32")
            nc.vector.reciprocal_approx_fast(out=r32, in_=sps)
            rr = small.tile([1, NT], BF16, tag="rr")
            nc.vector.tensor_copy(out=rr, in_=r32)
            rbps = ps_mix.tile([128, NT], F32, tag="mix")
            nc.tensor.matmul(rbps, lhsT=ones_p, rhs=rr, start=True, stop=True)
            rb_sb = sm.tile([128, NT], F32, tag="rbsb")
            nc.vector.tensor_copy(out=rb_sb, in_=rbps)

            # y -> SBUF (f32r) for the W conv
            y_sb = sm.tile([128, NT], BF16, tag="ysb")
            nc.vector.tensor_copy(out=y_sb, in_=yps)

            for ch in range(CCH):
                wps = ps_ft.tile([128, NT], F32, tag="ft")
                nc.tensor.matmul(wps, lhsT=ww_sb[:, ch, :], rhs=y_sb,
                                 start=True, stop=True)
                # wy_n = wps * rb  (fp16), fused BN s1 accumulation
                nc.vector.scalar_tensor_tensor(
                    out=wy16[ch][:, sl], in0=wps, scalar=1.0, in1=rb_sb,
                    op0=ALU.mult, op1=ALU.mult,
                    accum_out=s1p[:, ch, it:it + 1])
                # BN s2: square on gpsimd, reduce on vector
                sq = sm.tile([128, NT], F32, tag="sq")
                nc.vector.tensor_tensor(out=sq, in0=wy16[ch][:, sl],
                                        in1=wy16[ch][:, sl], op=ALU.mult)
                nc.vector.tensor_reduce(out=s2p[:, ch, it:it + 1], in_=sq,
                                        axis=AX.X, op=ALU.add)

        # ---- combine partials, AllReduce, finalize ----
        stats_sb = small.tile([128, 2 * CCH], F32, tag="stats")
        nc.vector.tensor_reduce(out=stats_sb[:, 0:2], in_=s1p[:, :, :],
                                axis=AX.X, op=ALU.add)
        nc.vector.tensor_reduce(out=stats_sb[:, 2:4], in_=s2p[:, :, :],
                                axis=AX.X, op=ALU.add)
        nc.sync.dma_start(out=stats_in[:, :], in_=stats_sb)
        nc.gpsimd.collective_compute(
            "AllReduce", ALU.add, replica_groups=[list(range(B))],
            ins=[stats_in[:, :]], outs=[stats_out[:, :]])
        stats_g = small.tile([128, 2 * CCH], F32, tag="statsg")
        nc.sync.dma_start(out=stats_g, in_=stats_out[:, :])

        out_sb = small.tile([128, CCH], F32, tag="outsb")
        mean2 = small.tile([128, CCH], F32, tag="fin")
        e22 = small.tile([128, CCH], F32, tag="fin")
        var2 = small.tile([128, CCH], F32, tag="fin")
        nc.vector.tensor_scalar_mul(out=mean2, in0=stats_g[:, 0:2],
                                    scalar1=INV_CNT)
        nc.vector.tensor_scalar_mul(out=e22, in0=stats_g[:, 2:4],
                                    scalar1=INV_CNT)
        m22 = small.tile([128, CCH], F32, tag="fin")
        nc.scalar.square(out=m22, in_=mean2)
        nc.vector.tensor_tensor(out=var2, in0=e22, in1=m22, op=ALU.subtract)
        sd2 = small.tile([128, CCH], F32, tag="fin")
        nc.scalar.activation(out=sd2, in_=var2, func=AF.Sqrt, bias=eps_sb,
                             scale=1.0)
        rstd2 = small.tile([128, CCH], F32, tag="fin")
        nc.vector.reciprocal_approx_fast(out=rstd2, in_=sd2)
        scale2 = small.tile([128, CCH], F32, tag="fin")
        nc.vector.tensor_tensor(out=scale2, in0=rstd2, in1=gamma_sb,
                                op=ALU.mult)
        ms2 = small.tile([128, CCH], F32, tag="fin")
        nc.vector.tensor_tensor(out=ms2, in0=mean2, in1=scale2, op=ALU.mult)
        negshift2 = small.tile([128, CCH], F32, tag="fin")
        nc.vector.tensor_tensor(out=negshift2, in0=ms2, in1=beta_sb,
                                op=ALU.subtract)
        for ch in range(CCH):
            # z = wy16*scale + x16 ; out = max_n z - negshift
            z = sm.tile([128, N], F16, tag=f"z{ch}")
            nc.vector.scalar_tensor_tensor(out=z, in0=wy16[ch][:, :],
                                           scalar=scale2[:, ch:ch + 1],
                                           in1=x16[ch][:, :], op0=ALU.mult,
                                           op1=ALU.add)
            mx = small.tile([128, 1], F32, tag="fin")
            nc.vector.tensor_reduce(out=mx, in_=z, axis=AX.X, op=ALU.max)
            nc.vector.tensor_tensor(out=out_sb[:, ch:ch + 1], in0=mx,
                                    in1=negshift2[:, ch:ch + 1],
                                    op=ALU.subtract)
        for ch in range(CCH):
            nc.sync.dma_start(out=out_d[ch, :].rearrange("(p one) -> p one", one=1),
                              in_=out_sb[:, ch:ch + 1])

    nc.compile()
    return nc


_LAST = {}


def _to_bf16(a):
    try:
        import ml_dtypes
        return np.ascontiguousarray(a.astype(ml_dtypes.bfloat16))
    except ImportError:
        import jax.numpy as jnp
        return np.ascontiguousarray(np.asarray(jnp.asarray(a, dtype=jnp.bfloat16)))


def kernel(**inputs):
    x = np.ascontiguousarray(inputs["x"], dtype=np.float32)      # (8, 256, 64, 64)
    Wg = np.asarray(inputs["Wg"], dtype=np.float32)
    bg = np.asarray(inputs["bg"], dtype=np.float32)
    Wt = np.asarray(inputs["Wt"], dtype=np.float32)
    bt = np.asarray(inputs["bt"], dtype=np.float32)
    Wp = np.asarray(inputs["Wp"], dtype=np.float32)
    bp = np.asarray(inputs["bp"], dtype=np.float32)
    Ww = np.asarray(inputs["Ww"], dtype=np.float32)
    bw = np.asarray(inputs["bw"], dtype=np.float32)
    gamma = np.asarray(inputs["gamma"], dtype=np.float32)
    beta = np.asarray(inputs["beta"], dtype=np.float32)

    if "nc" not in _CACHE:
        _CACHE["nc"] = _build()
    nc = _CACHE["nc"]

    shared = {
        "WtT": np.ascontiguousarray(Wt.T),
        "WpT": np.ascontiguousarray(Wp.T),
        "WgT": np.ascontiguousarray(Wg.T),
        "WwT": _to_bf16(np.ascontiguousarray(Ww.T)),
        "smalls": np.ascontiguousarray(np.concatenate([
            bt.reshape(CI, 1), bp.reshape(CI, 1), bg.reshape(CI, 1),
            gamma.reshape(CCH, 128).T, beta.reshape(CCH, 128).T,
        ], axis=1).astype(np.float32)),
        "ones_k": _to_bf16(np.ones((128, 1), dtype=np.float32)),
        "ones_p": _to_bf16(np.ones((1, 128), dtype=np.float32)),
    }
    in_maps = [dict(shared, x=np.ascontiguousarray(x[b].reshape(C, N)))
               for b in range(B)]
    import os
    trace = bool(int(os.environ.get("KERNEL_TRACE", "0")))
    res = run_bass_kernel_spmd(nc, in_maps, core_ids=list(range(B)), trace=trace)
    _LAST["res"] = res
    out = np.stack([np.asarray(res.results[b]["out"]).reshape(C) for b in range(B)])
    return out.reshape(B, C, 1, 1).astype(np.float32)


if __name__ == "__main__":
    pass
